# revision 9
# baseline (speedup 1.0000x reference)
"""Causal self-attention (B=2, T=2048, C=1024, H=16) on 8 TRN2 NeuronCores.

Sharding: core c handles batch b = c//4 and heads 4*(c%4) .. 4*(c%4)+3
(data-parallel over B, tensor-parallel over heads; full K/V for its heads
is computed locally from the core's QKV projection slice).

Per-core dataflow (all matmuls in float32r = full-rate TF32-like mode):
  - host passes xT = x[b].T [C,T], wqk = [Wq_h|Wk_h]^T [C,512],
    wv = [Wv_h0|0|...|Wv_h3|0]^T [C,260] (65-wide blocks, last col zero)
  - qT/kT [64,T] per head via projection matmuls (contraction c on partitions)
  - v [t,260] with a ones column appended per head (65th of each block)
  - head pairs (2p, 2p+1) share S^T tiles: s [k=128, 1024] = [S_even|S_odd],
    exp on ScalarE (scale=1/8 fused), causal mask on diagonal k-tiles via one
    gpsimd affine_select over a [128,2,512] view
  - y^T [65, 512] += V'.T @ P^T accumulated over k-tiles; row 64 = softmax
    denominators (from the ones column)
  - normalize: reciprocal_approx_fast + gpsimd partition_broadcast + multiply
  - DMA y^T[h] [64, T] out; host transposes/concats heads.

Pipelining: inputs are DMA'd in t-block slices and the emission order stages
projection chains immediately ahead of the attention q-blocks that consume
them, so TensorE stays dense from ~10us on and ScalarE (exp) starts early.
"""

import os
import sys
import types
import numpy as np

import concourse.bass as bass
import concourse.mybir as mybir
import concourse.tile as tile
from concourse import bacc
from concourse.bass_utils import run_bass_kernel_spmd

B, T, C, H = 2, 2048, 1024, 16
D = 64
NCORES = 8
HPC = 4          # heads per core
NQB = 4          # q blocks of 512
QB = 512
F32 = mybir.dt.float32
F32R = mybir.dt.float32r
EXP = mybir.ActivationFunctionType.Exp
MULT = mybir.AluOpType.mult
IS_GE = mybir.AluOpType.is_ge


def _install_profhook():
    """Register the NTFF profile hook shim so BASS_TRACE=1 works; harmless
    no-op (graceful trace skip) when the axon .so lacks profiling."""
    if "antenv.axon_hooks" not in sys.modules:
        mod = types.ModuleType("antenv.axon_hooks")
        mod._hook = None
        mod.set_axon_ntff_profile_hook = lambda h: setattr(mod, "_hook", h)
        mod.get_axon_ntff_profile_hook = lambda: mod._hook
        sys.modules["antenv.axon_hooks"] = mod
        try:
            import antenv
            antenv.axon_hooks = mod
        except ImportError:
            pass
    try:
        from trn_agent_boot.trn_boot import _ntff_profile_via_ctypes
        sys.modules["antenv.axon_hooks"].set_axon_ntff_profile_hook(
            _ntff_profile_via_ctypes("/opt/axon/libaxon_pjrt.so")
        )
        import concourse.bass_utils as bu
        bu.upload_artifacts = lambda tmpdir: tmpdir
    except Exception:
        pass


_install_profhook()

_NC = None


def _build():
    nc = bacc.Bacc("TRN2", target_bir_lowering=False, debug=False,
                   num_devices=NCORES)
    xT_d = nc.declare_dram_parameter("xT", [C, T], F32R, isOutput=False)
    wqk_d = nc.declare_dram_parameter("wqk", [C, 2 * HPC * D], F32R,
                                      isOutput=False)
    wv_d = nc.declare_dram_parameter("wv", [C, HPC * 65], F32R,
                                     isOutput=False)
    y_d = nc.declare_dram_parameter("y", [HPC, D, T], F32, isOutput=True)

    from contextlib import ExitStack
    with tile.TileContext(nc) as tc, ExitStack() as ctx:
        sb = ctx.enter_context(tc.tile_pool(name="sb", bufs=1))
        pp = ctx.enter_context(tc.tile_pool(name="pp", bufs=6))
        yp = ctx.enter_context(tc.tile_pool(name="yp", bufs=3))
        psp = ctx.enter_context(tc.tile_pool(name="psp", bufs=2, space="PSUM"))
        pss = ctx.enter_context(tc.tile_pool(name="pss", bufs=2, space="PSUM"))
        psy = ctx.enter_context(tc.tile_pool(name="psy", bufs=1, space="PSUM"))

        # per-(c, t-block) x^T tiles; per-t-block q/k tiles -> fine-grain deps
        xTt = [[sb.tile([128, 512], F32R, name=f"xT{c}_{tb}")
                for tb in range(4)] for c in range(8)]
        wqks = [sb.tile([128, 512], F32R, name=f"wqk{i}") for i in range(8)]
        wvs = [sb.tile([128, 260], F32R, name=f"wv{i}") for i in range(8)]
        qs = [[sb.tile([128, 512], F32R, name=f"q{p}_{tb}") for tb in range(4)]
              for p in range(2)]
        ks = [[sb.tile([128, 512], F32R, name=f"k{p}_{tb}") for tb in range(4)]
              for p in range(2)]
        vs = [sb.tile([128, 260], F32R, name=f"v_{t}") for t in range(16)]
        ones2 = sb.tile([128, 4], F32, name="ones2")
        nc.gpsimd.memset(ones2[:], 1.0)

        # DMA order: (wqk[c], xT[c,0]) pairs so the first projection chain can
        # start after ~2 transfers; then wv, then xT t-blocks 1..3
        for c in range(8):
            nc.sync.dma_start(wqks[c][:], wqk_d.ap()[c * 128:(c + 1) * 128, :])
            nc.sync.dma_start(xTt[c][0][:],
                              xT_d.ap()[c * 128:(c + 1) * 128, 0:512])
        for c in range(8):
            nc.sync.dma_start(wvs[c][:], wv_d.ap()[c * 128:(c + 1) * 128, :])
        for tb in range(1, 4):
            for c in range(8):
                nc.sync.dma_start(
                    xTt[c][tb][:],
                    xT_d.ap()[c * 128:(c + 1) * 128, tb * 512:(tb + 1) * 512])

        def qk_chain(p, ft_kind, tb):
            """One projection chain: q (ft_kind=0) or k (ft_kind=1) of pair p,
            t-block tb."""
            ft = p if ft_kind == 0 else 2 + p
            dst = (qs if ft_kind == 0 else ks)[p][tb]
            mm = psp.tile([128, 512], F32, name=f"pqk{p}_{ft}_{tb}", tag="pmm")
            for c in range(8):
                nc.tensor.matmul(mm[:],
                                 wqks[c][:, ft * 128:(ft + 1) * 128],
                                 xTt[c][tb][:],
                                 start=(c == 0), stop=(c == 7))
            nc.vector.tensor_copy(dst[:], mm[:])

        def v_chain(tt):
            """Combined v projection for one t-tile (all 4 heads, N=260)."""
            tb, sub = tt // 4, tt % 4
            mmv = psp.tile([128, 260], F32, name=f"pv{tt}", tag="pmm")
            for c in range(8):
                nc.tensor.matmul(mmv[:],
                                 xTt[c][tb][:, sub * 128:(sub + 1) * 128],
                                 wvs[c][:],
                                 start=(c == 0), stop=(c == 7))
            nc.vector.tensor_copy(vs[tt][:], mmv[:])
            nc.vector.tensor_copy(vs[tt][:, 64:260:65], ones2[:])

        def attn_chunks(p, j):
            """Chunk emitters for (pair p, q-block j): one per k-tile."""
            nkt = 4 * (j + 1)
            state = {}
            ops = []
            for kk in range(nkt):
                def emit(p=p, j=j, kk=kk, state=state, nkt=nkt):
                    if kk == 0:
                        state["ye"] = psy.tile([65, 512], F32,
                                               name=f"ye{p}_{j}", tag="ye")
                        state["yo"] = psy.tile([65, 512], F32,
                                               name=f"yo{p}_{j}", tag="yo")
                    s = pss.tile([128, 1024], F32,
                                 name=f"s{p}_{j}_{kk}", tag="s")
                    ktb, ksub = kk // 4, (kk % 4) * 128
                    with tc.tile_critical():
                        # adjacent 64-row matmuls on disjoint row groups
                        # (rows 0-63 / 64-127) pack concurrently in the array
                        nc.tensor.matmul(s[:, 0:512],
                                         ks[p][ktb][0:64, ksub:ksub + 128],
                                         qs[p][j][0:64, :],
                                         start=True, stop=True)
                        nc.tensor.matmul(s[:, 512:1024],
                                         ks[p][ktb][64:128, ksub:ksub + 128],
                                         qs[p][j][64:128, :],
                                         start=True, stop=True)
                    pt = pp.tile([128, 1024], F32R,
                                 name=f"pt{p}_{j}_{kk}", tag="pt")
                    nc.scalar.activation(pt[:], s[:], EXP, scale=0.125)
                    if kk >= 4 * j:
                        # causal mask both head halves in one op:
                        # [128, 2, 512] view, keep where q >= k
                        v3 = pt[:].rearrange("p (b q) -> p b q", b=2)
                        nc.gpsimd.affine_select(
                            v3, v3,
                            pattern=[[0, 2], [1, 512]],
                            compare_op=IS_GE, fill=0.0,
                            base=512 * j - 128 * kk,
                            channel_multiplier=-1)
                    first, last = (kk == 0), (kk == nkt - 1)
                    nc.tensor.matmul(state["ye"][:],
                                     vs[kk][:, 130 * p:130 * p + 65],
                                     pt[:, 0:512],
                                     start=first, stop=last)
                    nc.tensor.matmul(state["yo"][:],
                                     vs[kk][:, 130 * p + 65:130 * p + 130],
                                     pt[:, 512:1024],
                                     start=first, stop=last)
                    if last:
                        for h01, key in ((0, "ye"), (1, "yo")):
                            ysb = yp.tile([65, 512], F32,
                                          name=f"ysb{p}_{j}_{h01}", tag="ysb")
                            nc.vector.tensor_copy(ysb[:], state[key][:])
                            ssb = yp.tile([1, 512], F32,
                                          name=f"ssb{p}_{j}_{h01}", tag="ssb")
                            nc.vector.tensor_copy(ssb[:], ysb[64:65, :])
                            rsb = yp.tile([1, 512], F32,
                                          name=f"rsb{p}_{j}_{h01}", tag="rsb")
                            nc.vector.reciprocal_approx_fast(
                                out=rsb[:], in_=ssb[:])
                            bsb = yp.tile([64, 512], F32,
                                          name=f"bsb{p}_{j}_{h01}", tag="bsb")
                            nc.gpsimd.partition_broadcast(
                                bsb[:], rsb[:], channels=64)
                            yn = yp.tile([64, 512], F32,
                                         name=f"yn{p}_{j}_{h01}", tag="yn")
                            nc.vector.tensor_tensor(
                                yn[:], ysb[0:64, :], bsb[:], op=MULT)
                            nc.sync.dma_start(
                                y_d.ap()[2 * p + h01, :,
                                         j * 512:(j + 1) * 512],
                                yn[:])
                ops.append(emit)
            return ops

        # staged emission: per q-block j, pair-0 projections for t-block j,
        # the v tiles it needs, then attn0 q-block j with pair-1 projections
        # interleaved; attn1 runs last (its inputs are all ready by then).
        for j in range(NQB):
            qk_chain(0, 0, j)          # q pair0, t-block j
            qk_chain(0, 1, j)          # k pair0, t-block j
            for tt in range(4 * j, 4 * j + 4):
                v_chain(tt)
            chunks = attn_chunks(0, j)
            p1work = [lambda j=j: qk_chain(1, 0, j),
                      lambda j=j: qk_chain(1, 1, j)]
            k = 0
            for i, op in enumerate(chunks):
                op()
                tgt = (i + 1) * len(p1work) // len(chunks)
                while k < tgt:
                    p1work[k]()
                    k += 1
        for j in range(NQB):
            for op in attn_chunks(1, j):
                op()

    nc.compile()
    return nc


def _get_nc():
    global _NC
    if _NC is None:
        _NC = _build()
    return _NC


def _make_in_maps(x, W_attn):
    x = np.asarray(x, dtype=np.float32)
    W = np.asarray(W_attn, dtype=np.float32)
    wq, wk, wv = W[0:C], W[C:2 * C], W[2 * C:3 * C]
    in_maps = []
    for c in range(NCORES):
        b, g = c // 4, c % 4
        heads = [HPC * g + i for i in range(HPC)]
        xTb = np.ascontiguousarray(x[b].T)
        qrows = np.concatenate([wq[D * h:D * h + D] for h in heads], axis=0)
        krows = np.concatenate([wk[D * h:D * h + D] for h in heads], axis=0)
        wqk_np = np.ascontiguousarray(np.concatenate([qrows, krows], 0).T)
        wv_np = np.zeros((C, HPC * 65), np.float32)
        for i, h in enumerate(heads):
            wv_np[:, 65 * i:65 * i + D] = wv[D * h:D * h + D].T
        in_maps.append({"xT": xTb, "wqk": wqk_np, "wv": wv_np})
    return in_maps


def _execute(in_maps, trace=False):
    return run_bass_kernel_spmd(_get_nc(), in_maps,
                                core_ids=list(range(NCORES)), trace=trace)


def _assemble(results):
    y = np.empty((B, T, C), np.float32)
    for c in range(NCORES):
        b, g = c // 4, c % 4
        yc = results[c]["y"]
        for i in range(HPC):
            h = HPC * g + i
            y[b, :, D * h:D * h + D] = yc[i].T
    return y


def kernel(x, W_attn):
    res = _execute(_make_in_maps(x, W_attn), trace=False)
    return _assemble(res.results)


# revision 10
# speedup vs baseline: 1.5122x; 1.5122x over previous
"""Causal self-attention (B=2, T=2048, C=1024, H=16) on 8 TRN2 NeuronCores.

Sharding: core c handles batch b = c//4 and heads 4*(c%4) .. 4*(c%4)+3
(data-parallel over B, tensor-parallel over heads; full K/V for its heads
is computed locally from the core's QKV projection slice).

Per-core dataflow (all matmuls in float32r = full-rate TF32-like mode):
  - host passes xT = x[b].T [C,T], wqk = [Wq_h|Wk_h]^T [C,512],
    wv = [Wv_h0|0|...|Wv_h3|0]^T [C,260] (65-wide blocks, last col zero)
  - qT/kT [64,T] per head via projection matmuls (contraction c on partitions)
  - v [t,260] with a ones column appended per head (65th of each block)
  - head pairs (2p, 2p+1) share S^T tiles: s [k=128, 1024] = [S_even|S_odd],
    exp on ScalarE (scale=1/8 fused), causal mask on diagonal k-tiles via one
    gpsimd affine_select over a [128,2,512] view
  - y^T [65, 512] += V'.T @ P^T accumulated over k-tiles; row 64 = softmax
    denominators (from the ones column)
  - normalize: reciprocal_approx_fast + gpsimd partition_broadcast + multiply
  - DMA y^T[h] [64, T] out; host transposes/concats heads.

Pipelining: inputs are DMA'd in t-block slices and the emission order stages
projection chains immediately ahead of the attention q-blocks that consume
them, so TensorE stays dense from ~10us on and ScalarE (exp) starts early.
"""

import os
import sys
import types
import numpy as np

import concourse.bass as bass
import concourse.mybir as mybir
import concourse.tile as tile
from concourse import bacc
from concourse.bass_utils import run_bass_kernel_spmd

B, T, C, H = 2, 2048, 1024, 16
D = 64
NCORES = 8
HPC = 4          # heads per core
NQB = 4          # q blocks of 512
QB = 512
F32 = mybir.dt.float32
F32R = mybir.dt.float32r
EXP = mybir.ActivationFunctionType.Exp
MULT = mybir.AluOpType.mult
IS_GE = mybir.AluOpType.is_ge


def _install_profhook():
    """Register the NTFF profile hook shim so BASS_TRACE=1 works; harmless
    no-op (graceful trace skip) when the axon .so lacks profiling."""
    if "antenv.axon_hooks" not in sys.modules:
        mod = types.ModuleType("antenv.axon_hooks")
        mod._hook = None
        mod.set_axon_ntff_profile_hook = lambda h: setattr(mod, "_hook", h)
        mod.get_axon_ntff_profile_hook = lambda: mod._hook
        sys.modules["antenv.axon_hooks"] = mod
        try:
            import antenv
            antenv.axon_hooks = mod
        except ImportError:
            pass
    try:
        from trn_agent_boot.trn_boot import _ntff_profile_via_ctypes
        sys.modules["antenv.axon_hooks"].set_axon_ntff_profile_hook(
            _ntff_profile_via_ctypes("/opt/axon/libaxon_pjrt.so")
        )
        import concourse.bass_utils as bu
        bu.upload_artifacts = lambda tmpdir: tmpdir
    except Exception:
        pass


_install_profhook()

_NC = None


def _build():
    nc = bacc.Bacc("TRN2", target_bir_lowering=False, debug=False,
                   num_devices=NCORES)
    xT_d = nc.declare_dram_parameter("xT", [C, T], F32R, isOutput=False)
    wqk_d = nc.declare_dram_parameter("wqk", [C, 2 * HPC * D], F32R,
                                      isOutput=False)
    wv_d = nc.declare_dram_parameter("wv", [C, HPC * 65], F32R,
                                     isOutput=False)
    y_d = nc.declare_dram_parameter("y", [HPC, D, T], F32, isOutput=True)

    from contextlib import ExitStack
    with tile.TileContext(nc) as tc, ExitStack() as ctx:
        sb = ctx.enter_context(tc.tile_pool(name="sb", bufs=1))
        pp = ctx.enter_context(tc.tile_pool(name="pp", bufs=6))
        yp = ctx.enter_context(tc.tile_pool(name="yp", bufs=3))
        psp = ctx.enter_context(tc.tile_pool(name="psp", bufs=2, space="PSUM"))
        pss = ctx.enter_context(tc.tile_pool(name="pss", bufs=2, space="PSUM"))
        psy = ctx.enter_context(tc.tile_pool(name="psy", bufs=1, space="PSUM"))

        # per-(c, t-block) x^T tiles; per-t-block q/k tiles -> fine-grain deps
        xTt = [[sb.tile([128, 512], F32R, name=f"xT{c}_{tb}")
                for tb in range(4)] for c in range(8)]
        wqks = [sb.tile([128, 512], F32R, name=f"wqk{i}") for i in range(8)]
        wvs = [sb.tile([128, 260], F32R, name=f"wv{i}") for i in range(8)]
        qs = [[sb.tile([128, 512], F32R, name=f"q{p}_{tb}") for tb in range(4)]
              for p in range(2)]
        ks = [[sb.tile([128, 512], F32R, name=f"k{p}_{tb}") for tb in range(4)]
              for p in range(2)]
        vs = [sb.tile([128, 260], F32R, name=f"v_{t}") for t in range(16)]
        ones2 = sb.tile([128, 4], F32, name="ones2")
        nc.gpsimd.memset(ones2[:], 1.0)

        # DMA order: (wqk[c], xT[c,0]) pairs so the first projection chain can
        # start after ~2 transfers; then wv, then xT t-blocks 1..3
        for c in range(8):
            nc.sync.dma_start(wqks[c][:], wqk_d.ap()[c * 128:(c + 1) * 128, :])
            nc.sync.dma_start(xTt[c][0][:],
                              xT_d.ap()[c * 128:(c + 1) * 128, 0:512])
        for c in range(8):
            nc.sync.dma_start(wvs[c][:], wv_d.ap()[c * 128:(c + 1) * 128, :])
        for tb in range(1, 4):
            for c in range(8):
                nc.sync.dma_start(
                    xTt[c][tb][:],
                    xT_d.ap()[c * 128:(c + 1) * 128, tb * 512:(tb + 1) * 512])

        def qk_chain(p, ft_kind, tb):
            """One projection chain: q (ft_kind=0) or k (ft_kind=1) of pair p,
            t-block tb."""
            ft = p if ft_kind == 0 else 2 + p
            dst = (qs if ft_kind == 0 else ks)[p][tb]
            mm = psp.tile([128, 512], F32, name=f"pqk{p}_{ft}_{tb}", tag="pmm")
            for c in range(8):
                nc.tensor.matmul(mm[:],
                                 wqks[c][:, ft * 128:(ft + 1) * 128],
                                 xTt[c][tb][:],
                                 start=(c == 0), stop=(c == 7))
            nc.vector.tensor_copy(dst[:], mm[:])

        def v_chain(tt):
            """Combined v projection for one t-tile (all 4 heads, N=260)."""
            tb, sub = tt // 4, tt % 4
            mmv = psp.tile([128, 260], F32, name=f"pv{tt}", tag="pmm")
            for c in range(8):
                nc.tensor.matmul(mmv[:],
                                 xTt[c][tb][:, sub * 128:(sub + 1) * 128],
                                 wvs[c][:],
                                 start=(c == 0), stop=(c == 7))
            nc.vector.tensor_copy(vs[tt][:], mmv[:])
            nc.vector.tensor_copy(vs[tt][:, 64:260:65], ones2[:])

        def attn_chunks(p, j):
            """Chunk emitters for (pair p, q-block j): one per k-tile."""
            nkt = 4 * (j + 1)
            state = {}
            ops = []
            for kk in range(nkt):
                def emit(p=p, j=j, kk=kk, state=state, nkt=nkt):
                    if kk == 0:
                        state["ye"] = psy.tile([65, 512], F32,
                                               name=f"ye{p}_{j}", tag="ye")
                        state["yo"] = psy.tile([65, 512], F32,
                                               name=f"yo{p}_{j}", tag="yo")
                    s = pss.tile([128, 1024], F32,
                                 name=f"s{p}_{j}_{kk}", tag="s")
                    ktb, ksub = kk // 4, (kk % 4) * 128
                    nc.tensor.matmul(s[:, 0:512],
                                     ks[p][ktb][0:64, ksub:ksub + 128],
                                     qs[p][j][0:64, :],
                                     start=True, stop=True)
                    nc.tensor.matmul(s[:, 512:1024],
                                     ks[p][ktb][64:128, ksub:ksub + 128],
                                     qs[p][j][64:128, :],
                                     start=True, stop=True)
                    pt = pp.tile([128, 1024], F32R,
                                 name=f"pt{p}_{j}_{kk}", tag="pt")
                    nc.scalar.activation(pt[:], s[:], EXP, scale=0.125)
                    if kk >= 4 * j:
                        # causal mask both head halves in one op:
                        # [128, 2, 512] view, keep where q >= k
                        v3 = pt[:].rearrange("p (b q) -> p b q", b=2)
                        nc.gpsimd.affine_select(
                            v3, v3,
                            pattern=[[0, 2], [1, 512]],
                            compare_op=IS_GE, fill=0.0,
                            base=512 * j - 128 * kk,
                            channel_multiplier=-1)
                    first, last = (kk == 0), (kk == nkt - 1)
                    nc.tensor.matmul(state["ye"][:],
                                     vs[kk][:, 130 * p:130 * p + 65],
                                     pt[:, 0:512],
                                     start=first, stop=last)
                    nc.tensor.matmul(state["yo"][:],
                                     vs[kk][:, 130 * p + 65:130 * p + 130],
                                     pt[:, 512:1024],
                                     start=first, stop=last)
                    if last:
                        for h01, key in ((0, "ye"), (1, "yo")):
                            ysb = yp.tile([65, 512], F32,
                                          name=f"ysb{p}_{j}_{h01}", tag="ysb")
                            nc.vector.tensor_copy(ysb[:], state[key][:])
                            ssb = yp.tile([1, 512], F32,
                                          name=f"ssb{p}_{j}_{h01}", tag="ssb")
                            nc.vector.tensor_copy(ssb[:], ysb[64:65, :])
                            rsb = yp.tile([1, 512], F32,
                                          name=f"rsb{p}_{j}_{h01}", tag="rsb")
                            nc.vector.reciprocal_approx_fast(
                                out=rsb[:], in_=ssb[:])
                            bsb = yp.tile([64, 512], F32,
                                          name=f"bsb{p}_{j}_{h01}", tag="bsb")
                            nc.gpsimd.partition_broadcast(
                                bsb[:], rsb[:], channels=64)
                            yn = yp.tile([64, 512], F32,
                                         name=f"yn{p}_{j}_{h01}", tag="yn")
                            nc.vector.tensor_tensor(
                                yn[:], ysb[0:64, :], bsb[:], op=MULT)
                            nc.sync.dma_start(
                                y_d.ap()[2 * p + h01, :,
                                         j * 512:(j + 1) * 512],
                                yn[:])
                ops.append(emit)
            return ops

        # staged emission: per q-block j, pair-0 projections for t-block j,
        # the v tiles it needs, then attn0 q-block j with pair-1 projections
        # interleaved; attn1 runs last (its inputs are all ready by then).
        for j in range(NQB):
            qk_chain(0, 0, j)          # q pair0, t-block j
            qk_chain(0, 1, j)          # k pair0, t-block j
            for tt in range(4 * j, 4 * j + 4):
                v_chain(tt)
            chunks = attn_chunks(0, j)
            p1work = [lambda j=j: qk_chain(1, 0, j),
                      lambda j=j: qk_chain(1, 1, j)]
            k = 0
            for i, op in enumerate(chunks):
                op()
                tgt = (i + 1) * len(p1work) // len(chunks)
                while k < tgt:
                    p1work[k]()
                    k += 1
        for j in range(NQB):
            for op in attn_chunks(1, j):
                op()

    nc.compile()
    return nc


def _get_nc():
    global _NC
    if _NC is None:
        _NC = _build()
    return _NC


def _make_in_maps(x, W_attn):
    x = np.asarray(x, dtype=np.float32)
    W = np.asarray(W_attn, dtype=np.float32)
    wq, wk, wv = W[0:C], W[C:2 * C], W[2 * C:3 * C]
    in_maps = []
    for c in range(NCORES):
        b, g = c // 4, c % 4
        heads = [HPC * g + i for i in range(HPC)]
        xTb = np.ascontiguousarray(x[b].T)
        qrows = np.concatenate([wq[D * h:D * h + D] for h in heads], axis=0)
        krows = np.concatenate([wk[D * h:D * h + D] for h in heads], axis=0)
        wqk_np = np.ascontiguousarray(np.concatenate([qrows, krows], 0).T)
        wv_np = np.zeros((C, HPC * 65), np.float32)
        for i, h in enumerate(heads):
            wv_np[:, 65 * i:65 * i + D] = wv[D * h:D * h + D].T
        in_maps.append({"xT": xTb, "wqk": wqk_np, "wv": wv_np})
    return in_maps


def _execute(in_maps, trace=False):
    return run_bass_kernel_spmd(_get_nc(), in_maps,
                                core_ids=list(range(NCORES)), trace=trace)


def _assemble(results):
    y = np.empty((B, T, C), np.float32)
    for c in range(NCORES):
        b, g = c // 4, c % 4
        yc = results[c]["y"]
        for i in range(HPC):
            h = HPC * g + i
            y[b, :, D * h:D * h + D] = yc[i].T
    return y


def kernel(x, W_attn):
    res = _execute(_make_in_maps(x, W_attn), trace=False)
    return _assemble(res.results)


# revision 12
# speedup vs baseline: 1.5770x; 1.0428x over previous
"""Causal self-attention (B=2, T=2048, C=1024, H=16) on 8 TRN2 NeuronCores.

Sharding: core c handles batch b = c//4 and heads 4*(c%4) .. 4*(c%4)+3
(data-parallel over B, tensor-parallel over heads; full K/V for its heads
is computed locally from the core's QKV projection slice).

Per-core dataflow (all matmuls in float32r = full-rate TF32-like mode):
  - host passes xT = x[b].T [C,T], wqk = [Wq_h|Wk_h]^T [C,512],
    wv = [Wv_h0|0|...|Wv_h3|0]^T [C,260] (65-wide blocks, last col zero)
  - qT/kT [64,T] per head via projection matmuls (contraction c on partitions)
  - v [t,260] with a ones column appended per head (65th of each block)
  - head pairs (2p, 2p+1) share S^T tiles: s [k=128, 1024] = [S_even|S_odd],
    exp on ScalarE (scale=1/8 fused), causal mask on diagonal k-tiles via one
    gpsimd affine_select over a [128,2,512] view
  - y^T [65, 512] += V'.T @ P^T accumulated over k-tiles; row 64 = softmax
    denominators (from the ones column)
  - normalize: reciprocal_approx_fast + gpsimd partition_broadcast + multiply
  - DMA y^T[h] [64, T] out; host transposes/concats heads.

Pipelining: inputs are DMA'd in t-block slices and the emission order stages
projection chains immediately ahead of the attention q-blocks that consume
them, so TensorE stays dense from ~10us on and ScalarE (exp) starts early.
"""

import os
import sys
import types
import numpy as np

import concourse.bass as bass
import concourse.mybir as mybir
import concourse.tile as tile
from concourse import bacc
from concourse.bass_utils import run_bass_kernel_spmd

B, T, C, H = 2, 2048, 1024, 16
D = 64
NCORES = 8
HPC = 4          # heads per core
NQB = 4          # q blocks of 512
QB = 512
F32 = mybir.dt.float32
F32R = mybir.dt.float32r
EXP = mybir.ActivationFunctionType.Exp
MULT = mybir.AluOpType.mult
IS_GE = mybir.AluOpType.is_ge


def _install_profhook():
    """Register the NTFF profile hook shim so BASS_TRACE=1 works; harmless
    no-op (graceful trace skip) when the axon .so lacks profiling."""
    if "antenv.axon_hooks" not in sys.modules:
        mod = types.ModuleType("antenv.axon_hooks")
        mod._hook = None
        mod.set_axon_ntff_profile_hook = lambda h: setattr(mod, "_hook", h)
        mod.get_axon_ntff_profile_hook = lambda: mod._hook
        sys.modules["antenv.axon_hooks"] = mod
        try:
            import antenv
            antenv.axon_hooks = mod
        except ImportError:
            pass
    try:
        from trn_agent_boot.trn_boot import _ntff_profile_via_ctypes
        sys.modules["antenv.axon_hooks"].set_axon_ntff_profile_hook(
            _ntff_profile_via_ctypes("/opt/axon/libaxon_pjrt.so")
        )
        import concourse.bass_utils as bu
        bu.upload_artifacts = lambda tmpdir: tmpdir
    except Exception:
        pass


_install_profhook()

_NC = None


def _build():
    nc = bacc.Bacc("TRN2", target_bir_lowering=False, debug=False,
                   num_devices=NCORES)
    xT_d = nc.declare_dram_parameter("xT", [C, T], F32R, isOutput=False)
    wqk_d = nc.declare_dram_parameter("wqk", [C, 2 * HPC * D], F32R,
                                      isOutput=False)
    wv_d = nc.declare_dram_parameter("wv", [C, HPC * 65], F32R,
                                     isOutput=False)
    y_d = nc.declare_dram_parameter("y", [HPC, D, T], F32, isOutput=True)

    from contextlib import ExitStack
    with tile.TileContext(nc) as tc, ExitStack() as ctx:
        sb = ctx.enter_context(tc.tile_pool(name="sb", bufs=1))
        pp = ctx.enter_context(tc.tile_pool(name="pp", bufs=6))
        yp = ctx.enter_context(tc.tile_pool(name="yp", bufs=3))
        psp = ctx.enter_context(tc.tile_pool(name="psp", bufs=2, space="PSUM"))
        pss = ctx.enter_context(tc.tile_pool(name="pss", bufs=2, space="PSUM"))
        psy = ctx.enter_context(tc.tile_pool(name="psy", bufs=1, space="PSUM"))

        # per-(c, t-block) x^T tiles; per-t-block q/k tiles -> fine-grain deps
        xTt = [[sb.tile([128, 512], F32R, name=f"xT{c}_{tb}")
                for tb in range(4)] for c in range(8)]
        wqks = [sb.tile([128, 512], F32R, name=f"wqk{i}") for i in range(8)]
        wvs = [sb.tile([128, 260], F32R, name=f"wv{i}") for i in range(8)]
        qs = [[sb.tile([128, 512], F32R, name=f"q{p}_{tb}") for tb in range(4)]
              for p in range(2)]
        ks = [[sb.tile([128, 512], F32R, name=f"k{p}_{tb}") for tb in range(4)]
              for p in range(2)]
        vs = [sb.tile([128, 260], F32R, name=f"v_{t}") for t in range(16)]
        ones2 = sb.tile([128, 4], F32, name="ones2")
        nc.gpsimd.memset(ones2[:], 1.0)

        # DMA order: (wqk[c], xT[c,0]) pairs so the first projection chain can
        # start after ~2 transfers; then wv, then xT t-blocks 1..3
        for c in range(8):
            nc.sync.dma_start(wqks[c][:], wqk_d.ap()[c * 128:(c + 1) * 128, :])
            nc.sync.dma_start(xTt[c][0][:],
                              xT_d.ap()[c * 128:(c + 1) * 128, 0:512])
        for c in range(8):
            nc.sync.dma_start(wvs[c][:], wv_d.ap()[c * 128:(c + 1) * 128, :])
        for tb in range(1, 4):
            for c in range(8):
                nc.sync.dma_start(
                    xTt[c][tb][:],
                    xT_d.ap()[c * 128:(c + 1) * 128, tb * 512:(tb + 1) * 512])

        def qk_chain(p, ft_kind, tb):
            """One projection chain: q (ft_kind=0) or k (ft_kind=1) of pair p,
            t-block tb."""
            ft = p if ft_kind == 0 else 2 + p
            dst = (qs if ft_kind == 0 else ks)[p][tb]
            mm = psp.tile([128, 512], F32, name=f"pqk{p}_{ft}_{tb}", tag="pmm")
            for c in range(8):
                nc.tensor.matmul(mm[:],
                                 wqks[c][:, ft * 128:(ft + 1) * 128],
                                 xTt[c][tb][:],
                                 start=(c == 0), stop=(c == 7))
            nc.vector.tensor_copy(dst[:], mm[:])

        def v_chain(tt):
            """Combined v projection for one t-tile (all 4 heads, N=260)."""
            tb, sub = tt // 4, tt % 4
            mmv = psp.tile([128, 260], F32, name=f"pv{tt}", tag="pmm")
            for c in range(8):
                nc.tensor.matmul(mmv[:],
                                 xTt[c][tb][:, sub * 128:(sub + 1) * 128],
                                 wvs[c][:],
                                 start=(c == 0), stop=(c == 7))
            nc.vector.tensor_copy(vs[tt][:], mmv[:])
            nc.vector.tensor_copy(vs[tt][:, 64:260:65], ones2[:])

        def attn_s_part(p, j, kk, ptiles):
            """S matmuls + exp + causal mask for chunk (p, j, kk)."""
            s = pss.tile([128, 1024], F32, name=f"s{p}_{j}_{kk}", tag="s")
            ktb, ksub = kk // 4, (kk % 4) * 128
            nc.tensor.matmul(s[:, 0:512],
                             ks[p][ktb][0:64, ksub:ksub + 128],
                             qs[p][j][0:64, :],
                             start=True, stop=True)
            nc.tensor.matmul(s[:, 512:1024],
                             ks[p][ktb][64:128, ksub:ksub + 128],
                             qs[p][j][64:128, :],
                             start=True, stop=True)
            pt = pp.tile([128, 1024], F32R, name=f"pt{p}_{j}_{kk}", tag="pt")
            nc.scalar.activation(pt[:], s[:], EXP, scale=0.125)
            if kk >= 4 * j:
                # causal mask both head halves in one op:
                # [128, 2, 512] view, keep where q >= k
                v3 = pt[:].rearrange("p (b q) -> p b q", b=2)
                nc.gpsimd.affine_select(
                    v3, v3,
                    pattern=[[0, 2], [1, 512]],
                    compare_op=IS_GE, fill=0.0,
                    base=512 * j - 128 * kk,
                    channel_multiplier=-1)
            ptiles[(j, kk)] = pt

        def attn_pv_part(p, j, kk, state, ptiles):
            """PV accumulation (+ final normalize) for chunk (p, j, kk)."""
            nkt = 4 * (j + 1)
            if kk == 0:
                state["ye"] = psy.tile([65, 512], F32,
                                       name=f"ye{p}_{j}", tag="ye")
                state["yo"] = psy.tile([65, 512], F32,
                                       name=f"yo{p}_{j}", tag="yo")
            pt = ptiles.pop((j, kk))
            first, last = (kk == 0), (kk == nkt - 1)
            nc.tensor.matmul(state["ye"][:],
                             vs[kk][:, 130 * p:130 * p + 65],
                             pt[:, 0:512],
                             start=first, stop=last)
            nc.tensor.matmul(state["yo"][:],
                             vs[kk][:, 130 * p + 65:130 * p + 130],
                             pt[:, 512:1024],
                             start=first, stop=last)
            if last:
                for h01, key in ((0, "ye"), (1, "yo")):
                    ysb = yp.tile([65, 512], F32,
                                  name=f"ysb{p}_{j}_{h01}", tag="ysb")
                    nc.vector.tensor_copy(ysb[:], state[key][:])
                    ssb = yp.tile([1, 512], F32,
                                  name=f"ssb{p}_{j}_{h01}", tag="ssb")
                    nc.vector.tensor_copy(ssb[:], ysb[64:65, :])
                    rsb = yp.tile([1, 512], F32,
                                  name=f"rsb{p}_{j}_{h01}", tag="rsb")
                    nc.vector.reciprocal_approx_fast(out=rsb[:], in_=ssb[:])
                    bsb = yp.tile([64, 512], F32,
                                  name=f"bsb{p}_{j}_{h01}", tag="bsb")
                    nc.gpsimd.partition_broadcast(bsb[:], rsb[:], channels=64)
                    yn = yp.tile([64, 512], F32,
                                 name=f"yn{p}_{j}_{h01}", tag="yn")
                    nc.vector.tensor_tensor(yn[:], ysb[0:64, :], bsb[:],
                                            op=MULT)
                    nc.sync.dma_start(
                        y_d.ap()[2 * p + h01, :, j * 512:(j + 1) * 512],
                        yn[:])

        ptiles = {}
        states = {}

        def run_pair(p, stage_work):
            """Emit the pair's attention as one flat pipeline: the S/exp of
            chunk t is emitted before the PV of chunk t-1 (across q-block
            boundaries) so the in-order PE never stalls behind exp; per
            q-block stage_work (projection chains) is emitted up front."""
            seq = [(j, kk) for j in range(NQB) for kk in range(4 * (j + 1))]
            prev = None
            for (j, kk) in seq:
                if kk == 0:
                    for w in stage_work.get(j, ()):
                        w()
                attn_s_part(p, j, kk, ptiles)
                if prev is not None:
                    pj, pkk = prev
                    attn_pv_part(p, pj, pkk,
                                 states.setdefault((p, pj), {}), ptiles)
                prev = (j, kk)
            pj, pkk = prev
            attn_pv_part(p, pj, pkk, states.setdefault((p, pj), {}), ptiles)

        # stage j work: pair-0 projections for t-block j, the v tiles that
        # q-block j first needs, and pair-1 projections as PE filler
        stage0 = {}
        for j in range(NQB):
            work = [lambda j=j: qk_chain(0, 0, j),
                    lambda j=j: qk_chain(0, 1, j)]
            work += [lambda tt=tt: v_chain(tt)
                     for tt in range(4 * j, 4 * j + 4)]
            work += [lambda j=j: qk_chain(1, 0, j),
                     lambda j=j: qk_chain(1, 1, j)]
            stage0[j] = work
        run_pair(0, stage0)
        run_pair(1, {})

    nc.compile()
    return nc


def _get_nc():
    global _NC
    if _NC is None:
        _NC = _build()
    return _NC


def _make_in_maps(x, W_attn):
    x = np.asarray(x, dtype=np.float32)
    W = np.asarray(W_attn, dtype=np.float32)
    wq, wk, wv = W[0:C], W[C:2 * C], W[2 * C:3 * C]
    in_maps = []
    for c in range(NCORES):
        b, g = c // 4, c % 4
        heads = [HPC * g + i for i in range(HPC)]
        xTb = np.ascontiguousarray(x[b].T)
        qrows = np.concatenate([wq[D * h:D * h + D] for h in heads], axis=0)
        krows = np.concatenate([wk[D * h:D * h + D] for h in heads], axis=0)
        wqk_np = np.ascontiguousarray(np.concatenate([qrows, krows], 0).T)
        wv_np = np.zeros((C, HPC * 65), np.float32)
        for i, h in enumerate(heads):
            wv_np[:, 65 * i:65 * i + D] = wv[D * h:D * h + D].T
        in_maps.append({"xT": xTb, "wqk": wqk_np, "wv": wv_np})
    return in_maps


def _execute(in_maps, trace=False):
    return run_bass_kernel_spmd(_get_nc(), in_maps,
                                core_ids=list(range(NCORES)), trace=trace)


def _assemble(results):
    y = np.empty((B, T, C), np.float32)
    for c in range(NCORES):
        b, g = c // 4, c % 4
        yc = results[c]["y"]
        for i in range(HPC):
            h = HPC * g + i
            y[b, :, D * h:D * h + D] = yc[i].T
    return y


def kernel(x, W_attn):
    res = _execute(_make_in_maps(x, W_attn), trace=False)
    return _assemble(res.results)


# revision 14
# speedup vs baseline: 1.5847x; 1.0049x over previous
"""Causal self-attention (B=2, T=2048, C=1024, H=16) on 8 TRN2 NeuronCores.

Sharding: core c handles batch b = c//4 and heads 4*(c%4) .. 4*(c%4)+3
(data-parallel over B, tensor-parallel over heads; full K/V for its heads
is computed locally from the core's QKV projection slice).

Per-core dataflow (all matmuls in float32r = full-rate TF32-like mode):
  - host passes xT = x[b].T [C,T], wqk = [Wq_h|Wk_h]^T [C,512],
    wv = [Wv_h0|0|...|Wv_h3|0]^T [C,260] (65-wide blocks, last col zero)
  - qT/kT [64,T] per head via projection matmuls (contraction c on partitions)
  - v [t,260] with a ones column appended per head (65th of each block)
  - head pairs (2p, 2p+1) share S^T tiles: s [k=128, 1024] = [S_even|S_odd],
    exp on ScalarE (scale=1/8 fused), causal mask on diagonal k-tiles via one
    gpsimd affine_select over a [128,2,512] view
  - y^T [65, 512] += V'.T @ P^T accumulated over k-tiles; row 64 = softmax
    denominators (from the ones column)
  - normalize: reciprocal_approx_fast + gpsimd partition_broadcast + multiply
  - DMA y^T[h] [64, T] out; host transposes/concats heads.

Pipelining: inputs are DMA'd in t-block slices and the emission order stages
projection chains immediately ahead of the attention q-blocks that consume
them, so TensorE stays dense from ~10us on and ScalarE (exp) starts early.
"""

import os
import sys
import types
import numpy as np

import concourse.bass as bass
import concourse.mybir as mybir
import concourse.tile as tile
from concourse import bacc
from concourse.bass_utils import run_bass_kernel_spmd

B, T, C, H = 2, 2048, 1024, 16
D = 64
NCORES = 8
HPC = 4          # heads per core
NQB = 4          # q blocks of 512
QB = 512
F32 = mybir.dt.float32
F32R = mybir.dt.float32r
EXP = mybir.ActivationFunctionType.Exp
MULT = mybir.AluOpType.mult
IS_GE = mybir.AluOpType.is_ge


def _install_profhook():
    """Register the NTFF profile hook shim so BASS_TRACE=1 works; harmless
    no-op (graceful trace skip) when the axon .so lacks profiling."""
    if "antenv.axon_hooks" not in sys.modules:
        mod = types.ModuleType("antenv.axon_hooks")
        mod._hook = None
        mod.set_axon_ntff_profile_hook = lambda h: setattr(mod, "_hook", h)
        mod.get_axon_ntff_profile_hook = lambda: mod._hook
        sys.modules["antenv.axon_hooks"] = mod
        try:
            import antenv
            antenv.axon_hooks = mod
        except ImportError:
            pass
    try:
        from trn_agent_boot.trn_boot import _ntff_profile_via_ctypes
        sys.modules["antenv.axon_hooks"].set_axon_ntff_profile_hook(
            _ntff_profile_via_ctypes("/opt/axon/libaxon_pjrt.so")
        )
        import concourse.bass_utils as bu
        bu.upload_artifacts = lambda tmpdir: tmpdir
    except Exception:
        pass


_install_profhook()

_NC = None


def _build():
    nc = bacc.Bacc("TRN2", target_bir_lowering=False, debug=False,
                   num_devices=NCORES)
    xT_d = nc.declare_dram_parameter("xT", [C, T], F32R, isOutput=False)
    wqk_d = nc.declare_dram_parameter("wqk", [C, 2 * HPC * D], F32R,
                                      isOutput=False)
    wv_d = nc.declare_dram_parameter("wv", [C, HPC * 65], F32R,
                                     isOutput=False)
    y_d = nc.declare_dram_parameter("y", [HPC, D, T], F32, isOutput=True)

    from contextlib import ExitStack
    with tile.TileContext(nc) as tc, ExitStack() as ctx:
        sb = ctx.enter_context(tc.tile_pool(name="sb", bufs=1))
        pp = ctx.enter_context(tc.tile_pool(name="pp", bufs=6))
        yp = ctx.enter_context(tc.tile_pool(name="yp", bufs=3))
        psp = ctx.enter_context(tc.tile_pool(name="psp", bufs=2, space="PSUM"))
        pss = ctx.enter_context(tc.tile_pool(name="pss", bufs=2, space="PSUM"))
        psy = ctx.enter_context(tc.tile_pool(name="psy", bufs=1, space="PSUM"))

        # per-(c, t-block) x^T tiles; per-t-block q/k tiles -> fine-grain deps
        xTt = [[sb.tile([128, 512], F32R, name=f"xT{c}_{tb}")
                for tb in range(4)] for c in range(8)]
        wqks = [sb.tile([128, 512], F32R, name=f"wqk{i}") for i in range(8)]
        wvs = [sb.tile([128, 260], F32R, name=f"wv{i}") for i in range(8)]
        qs = [[sb.tile([128, 512], F32R, name=f"q{p}_{tb}") for tb in range(4)]
              for p in range(2)]
        ks = [[sb.tile([128, 512], F32R, name=f"k{p}_{tb}") for tb in range(4)]
              for p in range(2)]
        vs = [sb.tile([128, 260], F32R, name=f"v_{t}") for t in range(16)]
        ones2 = sb.tile([128, 4], F32, name="ones2")
        nc.gpsimd.memset(ones2[:], 1.0)

        # warm-up: keep the PE's HAM activity monitor busy while the input
        # DMAs land, so real matmuls start at 2.4 GHz instead of 1.2 GHz
        wupf = sb.tile([128, 512], F32, name="wupf")
        nc.gpsimd.memset(wupf[:], 0.5)
        wup = sb.tile([128, 512], F32R, name="wup")
        nc.vector.tensor_copy(wup[:], wupf[:])
        wups = psp.tile([128, 512], F32, name="wups", tag="pmm")
        for _ in range(30):
            nc.tensor.matmul(wups[:], wup[:, 0:128], wup[:], start=True,
                             stop=True)

        # DMA order: (wqk[c], xT[c,0]) pairs so the first projection chain can
        # start after ~2 transfers; then wv, then xT t-blocks 1..3
        for c in range(8):
            nc.sync.dma_start(wqks[c][:], wqk_d.ap()[c * 128:(c + 1) * 128, :])
            nc.sync.dma_start(xTt[c][0][:],
                              xT_d.ap()[c * 128:(c + 1) * 128, 0:512])
        for c in range(8):
            nc.sync.dma_start(wvs[c][:], wv_d.ap()[c * 128:(c + 1) * 128, :])
        for tb in range(1, 4):
            for c in range(8):
                nc.sync.dma_start(
                    xTt[c][tb][:],
                    xT_d.ap()[c * 128:(c + 1) * 128, tb * 512:(tb + 1) * 512])

        def qk_chain(p, ft_kind, tb):
            """One projection chain: q (ft_kind=0) or k (ft_kind=1) of pair p,
            t-block tb."""
            ft = p if ft_kind == 0 else 2 + p
            dst = (qs if ft_kind == 0 else ks)[p][tb]
            mm = psp.tile([128, 512], F32, name=f"pqk{p}_{ft}_{tb}", tag="pmm")
            for c in range(8):
                nc.tensor.matmul(mm[:],
                                 wqks[c][:, ft * 128:(ft + 1) * 128],
                                 xTt[c][tb][:],
                                 start=(c == 0), stop=(c == 7))
            nc.vector.tensor_copy(dst[:], mm[:])

        def v_chain(tt):
            """Combined v projection for one t-tile (all 4 heads, N=260)."""
            tb, sub = tt // 4, tt % 4
            mmv = psp.tile([128, 260], F32, name=f"pv{tt}", tag="pmm")
            for c in range(8):
                nc.tensor.matmul(mmv[:],
                                 xTt[c][tb][:, sub * 128:(sub + 1) * 128],
                                 wvs[c][:],
                                 start=(c == 0), stop=(c == 7))
            nc.vector.tensor_copy(vs[tt][:], mmv[:])
            nc.vector.tensor_copy(vs[tt][:, 64:260:65], ones2[:])

        def attn_s_part(p, j, kk, ptiles):
            """S matmuls + exp + causal mask for chunk (p, j, kk)."""
            s = pss.tile([128, 1024], F32, name=f"s{p}_{j}_{kk}", tag="s")
            ktb, ksub = kk // 4, (kk % 4) * 128
            nc.tensor.matmul(s[:, 0:512],
                             ks[p][ktb][0:64, ksub:ksub + 128],
                             qs[p][j][0:64, :],
                             start=True, stop=True)
            nc.tensor.matmul(s[:, 512:1024],
                             ks[p][ktb][64:128, ksub:ksub + 128],
                             qs[p][j][64:128, :],
                             start=True, stop=True)
            pt = pp.tile([128, 1024], F32R, name=f"pt{p}_{j}_{kk}", tag="pt")
            nc.scalar.activation(pt[:], s[:], EXP, scale=0.125)
            if kk >= 4 * j:
                # causal mask both head halves in one op:
                # [128, 2, 512] view, keep where q >= k
                v3 = pt[:].rearrange("p (b q) -> p b q", b=2)
                nc.gpsimd.affine_select(
                    v3, v3,
                    pattern=[[0, 2], [1, 512]],
                    compare_op=IS_GE, fill=0.0,
                    base=512 * j - 128 * kk,
                    channel_multiplier=-1)
            ptiles[(j, kk)] = pt

        def attn_pv_part(p, j, kk, state, ptiles):
            """PV accumulation (+ final normalize) for chunk (p, j, kk)."""
            nkt = 4 * (j + 1)
            if kk == 0:
                state["ye"] = psy.tile([65, 512], F32,
                                       name=f"ye{p}_{j}", tag="ye")
                state["yo"] = psy.tile([65, 512], F32,
                                       name=f"yo{p}_{j}", tag="yo")
            pt = ptiles.pop((j, kk))
            first, last = (kk == 0), (kk == nkt - 1)
            nc.tensor.matmul(state["ye"][:],
                             vs[kk][:, 130 * p:130 * p + 65],
                             pt[:, 0:512],
                             start=first, stop=last)
            nc.tensor.matmul(state["yo"][:],
                             vs[kk][:, 130 * p + 65:130 * p + 130],
                             pt[:, 512:1024],
                             start=first, stop=last)
            if last:
                for h01, key in ((0, "ye"), (1, "yo")):
                    ysb = yp.tile([65, 512], F32,
                                  name=f"ysb{p}_{j}_{h01}", tag="ysb")
                    nc.vector.tensor_copy(ysb[:], state[key][:])
                    ssb = yp.tile([1, 512], F32,
                                  name=f"ssb{p}_{j}_{h01}", tag="ssb")
                    nc.vector.tensor_copy(ssb[:], ysb[64:65, :])
                    rsb = yp.tile([1, 512], F32,
                                  name=f"rsb{p}_{j}_{h01}", tag="rsb")
                    nc.vector.reciprocal_approx_fast(out=rsb[:], in_=ssb[:])
                    bsb = yp.tile([64, 512], F32,
                                  name=f"bsb{p}_{j}_{h01}", tag="bsb")
                    nc.gpsimd.partition_broadcast(bsb[:], rsb[:], channels=64)
                    yn = yp.tile([64, 512], F32,
                                 name=f"yn{p}_{j}_{h01}", tag="yn")
                    nc.vector.tensor_tensor(yn[:], ysb[0:64, :], bsb[:],
                                            op=MULT)
                    nc.sync.dma_start(
                        y_d.ap()[2 * p + h01, :, j * 512:(j + 1) * 512],
                        yn[:])

        ptiles = {}
        states = {}

        def run_pair(p, stage_work):
            """Emit the pair's attention as one flat pipeline: the S/exp of
            chunk t is emitted before the PV of chunk t-1 (across q-block
            boundaries) so the in-order PE never stalls behind exp; per
            q-block stage_work (projection chains) is emitted up front."""
            seq = [(j, kk) for j in range(NQB) for kk in range(4 * (j + 1))]
            prev = None
            for (j, kk) in seq:
                if kk == 0:
                    for w in stage_work.get(j, ()):
                        w()
                attn_s_part(p, j, kk, ptiles)
                if prev is not None:
                    pj, pkk = prev
                    attn_pv_part(p, pj, pkk,
                                 states.setdefault((p, pj), {}), ptiles)
                prev = (j, kk)
            pj, pkk = prev
            attn_pv_part(p, pj, pkk, states.setdefault((p, pj), {}), ptiles)

        # stage j work: pair-0 projections for t-block j, the v tiles that
        # q-block j first needs, and pair-1 projections as PE filler
        stage0 = {}
        for j in range(NQB):
            work = [lambda j=j: qk_chain(0, 0, j),
                    lambda j=j: qk_chain(0, 1, j)]
            work += [lambda tt=tt: v_chain(tt)
                     for tt in range(4 * j, 4 * j + 4)]
            work += [lambda j=j: qk_chain(1, 0, j),
                     lambda j=j: qk_chain(1, 1, j)]
            stage0[j] = work
        run_pair(0, stage0)
        run_pair(1, {})

    nc.compile()
    return nc


def _get_nc():
    global _NC
    if _NC is None:
        _NC = _build()
    return _NC


def _make_in_maps(x, W_attn):
    x = np.asarray(x, dtype=np.float32)
    W = np.asarray(W_attn, dtype=np.float32)
    wq, wk, wv = W[0:C], W[C:2 * C], W[2 * C:3 * C]
    in_maps = []
    for c in range(NCORES):
        b, g = c // 4, c % 4
        heads = [HPC * g + i for i in range(HPC)]
        xTb = np.ascontiguousarray(x[b].T)
        qrows = np.concatenate([wq[D * h:D * h + D] for h in heads], axis=0)
        krows = np.concatenate([wk[D * h:D * h + D] for h in heads], axis=0)
        wqk_np = np.ascontiguousarray(np.concatenate([qrows, krows], 0).T)
        wv_np = np.zeros((C, HPC * 65), np.float32)
        for i, h in enumerate(heads):
            wv_np[:, 65 * i:65 * i + D] = wv[D * h:D * h + D].T
        in_maps.append({"xT": xTb, "wqk": wqk_np, "wv": wv_np})
    return in_maps


def _execute(in_maps, trace=False):
    return run_bass_kernel_spmd(_get_nc(), in_maps,
                                core_ids=list(range(NCORES)), trace=trace)


def _assemble(results):
    y = np.empty((B, T, C), np.float32)
    for c in range(NCORES):
        b, g = c // 4, c % 4
        yc = results[c]["y"]
        for i in range(HPC):
            h = HPC * g + i
            y[b, :, D * h:D * h + D] = yc[i].T
    return y


def kernel(x, W_attn):
    res = _execute(_make_in_maps(x, W_attn), trace=False)
    return _assemble(res.results)


# revision 16
# speedup vs baseline: 1.6103x; 1.0161x over previous
"""Causal self-attention (B=2, T=2048, C=1024, H=16) on 8 TRN2 NeuronCores.

Sharding: core c handles batch b = c//4 and heads 4*(c%4) .. 4*(c%4)+3
(data-parallel over B, tensor-parallel over heads; full K/V for its heads
is computed locally from the core's QKV projection slice).

Per-core dataflow (all matmuls in float32r = full-rate TF32-like mode):
  - host passes xT = x[b].T [C,T], wqk = [Wq_h|Wk_h]^T [C,512],
    wv = [Wv_h0|0|...|Wv_h3|0]^T [C,260] (65-wide blocks, last col zero)
  - qT/kT [64,T] per head via projection matmuls (contraction c on partitions)
  - v [t,260] with a ones column appended per head (65th of each block)
  - head pairs (2p, 2p+1) share S^T tiles: s [k=128, 1024] = [S_even|S_odd],
    exp on ScalarE (scale=1/8 fused), causal mask on diagonal k-tiles via one
    gpsimd affine_select over a [128,2,512] view
  - y^T [65, 512] += V'.T @ P^T accumulated over k-tiles; row 64 = softmax
    denominators (from the ones column)
  - normalize: reciprocal_approx_fast + gpsimd partition_broadcast + multiply
  - DMA y^T[h] [64, T] out; host transposes/concats heads.

Pipelining: inputs are DMA'd in t-block slices and the emission order stages
projection chains immediately ahead of the attention q-blocks that consume
them, so TensorE stays dense from ~10us on and ScalarE (exp) starts early.
"""

import os
import sys
import types
import numpy as np

import concourse.bass as bass
import concourse.mybir as mybir
import concourse.tile as tile
from concourse import bacc
from concourse.bass_utils import run_bass_kernel_spmd

B, T, C, H = 2, 2048, 1024, 16
D = 64
NCORES = 8
HPC = 4          # heads per core
NQB = 4          # q blocks of 512
QB = 512
F32 = mybir.dt.float32
F32R = mybir.dt.float32r
EXP = mybir.ActivationFunctionType.Exp
MULT = mybir.AluOpType.mult
IS_GE = mybir.AluOpType.is_ge


def _install_profhook():
    """Register the NTFF profile hook shim so BASS_TRACE=1 works; harmless
    no-op (graceful trace skip) when the axon .so lacks profiling."""
    if "antenv.axon_hooks" not in sys.modules:
        mod = types.ModuleType("antenv.axon_hooks")
        mod._hook = None
        mod.set_axon_ntff_profile_hook = lambda h: setattr(mod, "_hook", h)
        mod.get_axon_ntff_profile_hook = lambda: mod._hook
        sys.modules["antenv.axon_hooks"] = mod
        try:
            import antenv
            antenv.axon_hooks = mod
        except ImportError:
            pass
    try:
        from trn_agent_boot.trn_boot import _ntff_profile_via_ctypes
        sys.modules["antenv.axon_hooks"].set_axon_ntff_profile_hook(
            _ntff_profile_via_ctypes("/opt/axon/libaxon_pjrt.so")
        )
        import concourse.bass_utils as bu
        bu.upload_artifacts = lambda tmpdir: tmpdir
    except Exception:
        pass


_install_profhook()

_NC = None


def _build():
    nc = bacc.Bacc("TRN2", target_bir_lowering=False, debug=False,
                   num_devices=NCORES)
    xT_d = nc.declare_dram_parameter("xT", [C, T], F32R, isOutput=False)
    wqk_d = nc.declare_dram_parameter("wqk", [C, 2 * HPC * D], F32R,
                                      isOutput=False)
    wv_d = nc.declare_dram_parameter("wv", [C, HPC * 65], F32R,
                                     isOutput=False)
    y_d = nc.declare_dram_parameter("y", [HPC, D, T], F32, isOutput=True)

    from contextlib import ExitStack
    with tile.TileContext(nc) as tc, ExitStack() as ctx:
        sb = ctx.enter_context(tc.tile_pool(name="sb", bufs=1))
        pp = ctx.enter_context(tc.tile_pool(name="pp", bufs=6))
        yp = ctx.enter_context(tc.tile_pool(name="yp", bufs=3))
        psp = ctx.enter_context(tc.tile_pool(name="psp", bufs=2, space="PSUM"))
        pss = ctx.enter_context(tc.tile_pool(name="pss", bufs=2, space="PSUM"))
        psy = ctx.enter_context(tc.tile_pool(name="psy", bufs=1, space="PSUM"))

        # per-(c, t-block) x^T tiles; per-t-block q/k tiles -> fine-grain deps
        xTt = [[sb.tile([128, 512], F32R, name=f"xT{c}_{tb}")
                for tb in range(4)] for c in range(8)]
        wqks = [sb.tile([128, 512], F32R, name=f"wqk{i}") for i in range(8)]
        wvs = [sb.tile([128, 260], F32R, name=f"wv{i}") for i in range(8)]
        qs = [[sb.tile([128, 512], F32R, name=f"q{p}_{tb}") for tb in range(4)]
              for p in range(2)]
        ks = [[sb.tile([128, 512], F32R, name=f"k{p}_{tb}") for tb in range(4)]
              for p in range(2)]
        vs = [sb.tile([128, 260], F32R, name=f"v_{t}") for t in range(16)]
        ones2 = sb.tile([128, 4], F32, name="ones2")
        nc.gpsimd.memset(ones2[:], 1.0)

        # warm-up: keep the PE's HAM activity monitor busy while the input
        # DMAs land, so real matmuls start at 2.4 GHz instead of 1.2 GHz
        wupf = sb.tile([128, 512], F32, name="wupf")
        nc.gpsimd.memset(wupf[:], 0.5)
        wup = sb.tile([128, 512], F32R, name="wup")
        nc.vector.tensor_copy(wup[:], wupf[:])
        wups = psp.tile([128, 512], F32, name="wups", tag="pmm")
        for _ in range(30):
            nc.tensor.matmul(wups[:], wup[:, 0:128], wup[:], start=True,
                             stop=True)

        # DMA order: (wqk[c], xT[c,0]) pairs so the first projection chain can
        # start after ~2 transfers; then wv, then xT t-blocks 1..3
        for c in range(8):
            nc.sync.dma_start(wqks[c][:], wqk_d.ap()[c * 128:(c + 1) * 128, :])
            nc.sync.dma_start(xTt[c][0][:],
                              xT_d.ap()[c * 128:(c + 1) * 128, 0:512])
        for c in range(8):
            nc.sync.dma_start(wvs[c][:], wv_d.ap()[c * 128:(c + 1) * 128, :])
        for tb in range(1, 4):
            for c in range(8):
                nc.sync.dma_start(
                    xTt[c][tb][:],
                    xT_d.ap()[c * 128:(c + 1) * 128, tb * 512:(tb + 1) * 512])

        def qk_chain(p, ft_kind, tb):
            """One projection chain: q (ft_kind=0) or k (ft_kind=1) of pair p,
            t-block tb."""
            ft = p if ft_kind == 0 else 2 + p
            dst = (qs if ft_kind == 0 else ks)[p][tb]
            mm = psp.tile([128, 512], F32, name=f"pqk{p}_{ft}_{tb}", tag="pmm")
            for c in range(8):
                nc.tensor.matmul(mm[:],
                                 wqks[c][:, ft * 128:(ft + 1) * 128],
                                 xTt[c][tb][:],
                                 start=(c == 0), stop=(c == 7))
            nc.vector.tensor_copy(dst[:], mm[:])

        def v_chain(tt):
            """Combined v projection for one t-tile (all 4 heads, N=260)."""
            tb, sub = tt // 4, tt % 4
            mmv = psp.tile([128, 260], F32, name=f"pv{tt}", tag="pmm")
            for c in range(8):
                nc.tensor.matmul(mmv[:],
                                 xTt[c][tb][:, sub * 128:(sub + 1) * 128],
                                 wvs[c][:],
                                 start=(c == 0), stop=(c == 7))
            nc.vector.tensor_copy(vs[tt][:], mmv[:])
            nc.vector.tensor_copy(vs[tt][:, 64:260:65], ones2[:])

        def attn_s_part(p, j, kk, ptiles):
            """S matmuls + exp + causal mask for chunk (p, j, kk).

            Diagonal k-tiles only have valid scores for q >= k, i.e. local
            q >= off = 128*(kk-4j); the matmuls skip the dead columns and
            affine_select's fill overwrites them (including stale PSUM) with
            zeros."""
            off = max(0, 128 * (kk - 4 * j))
            s = pss.tile([128, 1024], F32, name=f"s{p}_{j}_{kk}", tag="s")
            ktb, ksub = kk // 4, (kk % 4) * 128
            nc.tensor.matmul(s[:, off:512],
                             ks[p][ktb][0:64, ksub:ksub + 128],
                             qs[p][j][0:64, off:512],
                             start=True, stop=True)
            nc.tensor.matmul(s[:, 512 + off:1024],
                             ks[p][ktb][64:128, ksub:ksub + 128],
                             qs[p][j][64:128, off:512],
                             start=True, stop=True)
            pt = pp.tile([128, 1024], F32R, name=f"pt{p}_{j}_{kk}", tag="pt")
            nc.scalar.activation(pt[:], s[:], EXP, scale=0.125)
            if kk >= 4 * j:
                # causal mask both head halves in one op:
                # [128, 2, 512] view, keep where q >= k
                v3 = pt[:].rearrange("p (b q) -> p b q", b=2)
                nc.gpsimd.affine_select(
                    v3, v3,
                    pattern=[[0, 2], [1, 512]],
                    compare_op=IS_GE, fill=0.0,
                    base=512 * j - 128 * kk,
                    channel_multiplier=-1)
            ptiles[(j, kk)] = pt

        def attn_pv_part(p, j, kk, state, ptiles):
            """PV accumulation (+ final normalize) for chunk (p, j, kk)."""
            nkt = 4 * (j + 1)
            if kk == 0:
                state["ye"] = psy.tile([65, 512], F32,
                                       name=f"ye{p}_{j}", tag="ye")
                state["yo"] = psy.tile([65, 512], F32,
                                       name=f"yo{p}_{j}", tag="yo")
            pt = ptiles.pop((j, kk))
            first, last = (kk == 0), (kk == nkt - 1)
            # skip columns where P is all-zero (above the causal diagonal);
            # their y contribution is zero and PSUM keeps the prior partials
            off = 0 if first else max(0, 128 * (kk - 4 * j))
            nc.tensor.matmul(state["ye"][:, off:512],
                             vs[kk][:, 130 * p:130 * p + 65],
                             pt[:, off:512],
                             start=first, stop=last)
            nc.tensor.matmul(state["yo"][:, off:512],
                             vs[kk][:, 130 * p + 65:130 * p + 130],
                             pt[:, 512 + off:1024],
                             start=first, stop=last)
            if last:
                for h01, key in ((0, "ye"), (1, "yo")):
                    ysb = yp.tile([65, 512], F32,
                                  name=f"ysb{p}_{j}_{h01}", tag="ysb")
                    nc.vector.tensor_copy(ysb[:], state[key][:])
                    ssb = yp.tile([1, 512], F32,
                                  name=f"ssb{p}_{j}_{h01}", tag="ssb")
                    nc.vector.tensor_copy(ssb[:], ysb[64:65, :])
                    rsb = yp.tile([1, 512], F32,
                                  name=f"rsb{p}_{j}_{h01}", tag="rsb")
                    nc.vector.reciprocal_approx_fast(out=rsb[:], in_=ssb[:])
                    bsb = yp.tile([64, 512], F32,
                                  name=f"bsb{p}_{j}_{h01}", tag="bsb")
                    nc.gpsimd.partition_broadcast(bsb[:], rsb[:], channels=64)
                    yn = yp.tile([64, 512], F32,
                                 name=f"yn{p}_{j}_{h01}", tag="yn")
                    nc.vector.tensor_tensor(yn[:], ysb[0:64, :], bsb[:],
                                            op=MULT)
                    nc.sync.dma_start(
                        y_d.ap()[2 * p + h01, :, j * 512:(j + 1) * 512],
                        yn[:])

        ptiles = {}
        states = {}

        def run_pair(p, stage_work):
            """Emit the pair's attention as one flat pipeline: the S/exp of
            chunk t is emitted before the PV of chunk t-1 (across q-block
            boundaries) so the in-order PE never stalls behind exp; per
            q-block stage_work (projection chains) is emitted up front."""
            seq = [(j, kk) for j in range(NQB) for kk in range(4 * (j + 1))]
            prev = None
            for (j, kk) in seq:
                if kk == 0:
                    for w in stage_work.get(j, ()):
                        w()
                attn_s_part(p, j, kk, ptiles)
                if prev is not None:
                    pj, pkk = prev
                    attn_pv_part(p, pj, pkk,
                                 states.setdefault((p, pj), {}), ptiles)
                prev = (j, kk)
            pj, pkk = prev
            attn_pv_part(p, pj, pkk, states.setdefault((p, pj), {}), ptiles)

        # stage j work: pair-0 projections for t-block j, the v tiles that
        # q-block j first needs, and pair-1 projections as PE filler
        stage0 = {}
        for j in range(NQB):
            work = [lambda j=j: qk_chain(0, 0, j),
                    lambda j=j: qk_chain(0, 1, j)]
            work += [lambda tt=tt: v_chain(tt)
                     for tt in range(4 * j, 4 * j + 4)]
            work += [lambda j=j: qk_chain(1, 0, j),
                     lambda j=j: qk_chain(1, 1, j)]
            stage0[j] = work
        run_pair(0, stage0)
        run_pair(1, {})

    nc.compile()
    return nc


def _get_nc():
    global _NC
    if _NC is None:
        _NC = _build()
    return _NC


def _make_in_maps(x, W_attn):
    x = np.asarray(x, dtype=np.float32)
    W = np.asarray(W_attn, dtype=np.float32)
    wq, wk, wv = W[0:C], W[C:2 * C], W[2 * C:3 * C]
    in_maps = []
    for c in range(NCORES):
        b, g = c // 4, c % 4
        heads = [HPC * g + i for i in range(HPC)]
        xTb = np.ascontiguousarray(x[b].T)
        qrows = np.concatenate([wq[D * h:D * h + D] for h in heads], axis=0)
        krows = np.concatenate([wk[D * h:D * h + D] for h in heads], axis=0)
        wqk_np = np.ascontiguousarray(np.concatenate([qrows, krows], 0).T)
        wv_np = np.zeros((C, HPC * 65), np.float32)
        for i, h in enumerate(heads):
            wv_np[:, 65 * i:65 * i + D] = wv[D * h:D * h + D].T
        in_maps.append({"xT": xTb, "wqk": wqk_np, "wv": wv_np})
    return in_maps


def _execute(in_maps, trace=False):
    return run_bass_kernel_spmd(_get_nc(), in_maps,
                                core_ids=list(range(NCORES)), trace=trace)


def _assemble(results):
    y = np.empty((B, T, C), np.float32)
    for c in range(NCORES):
        b, g = c // 4, c % 4
        yc = results[c]["y"]
        for i in range(HPC):
            h = HPC * g + i
            y[b, :, D * h:D * h + D] = yc[i].T
    return y


def kernel(x, W_attn):
    res = _execute(_make_in_maps(x, W_attn), trace=False)
    return _assemble(res.results)
